# revision 13
# baseline (speedup 1.0000x reference)
"""GuidedFilter (r=15, eps=0.5) Trainium2 Bass kernel.

Full inputs: guide, input_map [16,1,1024,1024] f32. Data-parallel over 8
NeuronCores (2 images/core). Host converts inputs to bf16 (halves HBM
traffic); output returned bf16 and upcast on host.

Per image, box(x) = Hpass(Vpass(x)):
  - H direction: running-sum scans (tensor_tensor_scan) split across the
    DVE and Pool engines. The three derived tensors (p, I*p, I*I) share ONE
    merged padded buffer so one scan instruction covers all three blocks
    (the recurrence telescopes across block boundaries; boundary outputs
    are garbage but never read).
  - V direction: PE band matmuls with constant bf16 weights (reflect folded
    into the band blocks), fp32 PSUM accumulate.
  - Elementwise chain in bf16 (DVE 2x mode) with ACT doing PSUM
    evacuations (1/961 normalization folded into evac scales) and Pool
    taking scalar_tensor_tensor ops. Works with a' = -a so no extra
    negation passes are needed (out = mean_b - mean_a'*I).
"""

import numpy as np
import ml_dtypes

R = 15
K = 2 * R + 1  # 31
EPS = 0.5
NORM = 1.0 / (K * K)  # 1/961

_CACHE = {}


def _build_band_weights(Hc, NT):
    """Wf[k, m] = weight of input row k in output row m's reflect window."""
    Wf = np.zeros((Hc, Hc), np.float32)
    for m in range(Hc):
        for t in range(m - R, m + R + 1):
            k = t
            if k < 0:
                k = -k
            if k > Hc - 1:
                k = 2 * (Hc - 1) - k
            Wf[k, m] += 1.0
    # Pack per out-tile j into [128, 3*128]:
    #   cols 0:128   = center block  (in-tile j,   K=128)
    #   cols 128:256 = top edge      (in-tile j-1 rows 113:128, K=15, rows 64:128)
    #   cols 256:384 = bottom edge   (in-tile j+1 rows 0:15,    K=15, rows 0:15)
    wv = np.zeros((NT, 128, 384), np.float32)
    for j in range(NT):
        r0 = j * 128
        wv[j, :, 0:128] = Wf[r0 : r0 + 128, r0 : r0 + 128]
        if j > 0:
            wv[j, 64:128, 128:256] = Wf[r0 - 64 : r0, r0 : r0 + 128]
        if j < NT - 1:
            wv[j, 0:15, 256:384] = Wf[r0 + 128 : r0 + 143, r0 : r0 + 128]
    return wv.astype(ml_dtypes.bfloat16)


def build_nc(n_img, Hc, Wc):
    """Build the Bass module for one core processing n_img images of [Hc, Wc]."""
    import concourse.bass as bass
    import concourse.tile as tile
    from concourse import bacc, mybir

    P = 128
    NT = Hc // P
    BS = Wc + 32          # padded block stride; interior at cols 16..16+Wc
    CH = 512              # psum chunk width
    NC_ = Wc // CH        # chunks per tile
    f32 = mybir.dt.float32
    bf16 = mybir.dt.bfloat16
    AX = mybir.AxisListType.X
    OP = mybir.AluOpType
    AF = mybir.ActivationFunctionType

    nc = bacc.Bacc("TRN2", target_bir_lowering=False, debug=False)
    g_dram = nc.dram_tensor("guide", [n_img, Hc, Wc], bf16, kind="ExternalInput")
    p_dram = nc.dram_tensor("input_map", [n_img, Hc, Wc], bf16, kind="ExternalInput")
    wv_dram = nc.dram_tensor("wv", [NT, 128, 384], bf16, kind="ExternalInput")
    o_dram = nc.dram_tensor("out", [n_img, Hc, Wc], bf16, kind="ExternalOutput")
    gap, pap, wap, oap = g_dram.ap(), p_dram.ap(), wv_dram.ap(), o_dram.ap()

    with tile.TileContext(nc) as tc:
        wpool = tc.alloc_tile_pool(name="wv", bufs=1)
        wv_sb = []

        xpi_pool = tc.alloc_tile_pool(name="xpi", bufs=NT + 1)   # guide, image-long
        xp3_pool = tc.alloc_tile_pool(name="xp3", bufs=3)        # p|Ip|II merged
        h_pool = tc.alloc_tile_pool(name="hx", bufs=5)           # hI + h3
        cf_pool = tc.alloc_tile_pool(name="cf", bufs=2)          # chain transients
        ab_pool = tc.alloc_tile_pool(name="ab", bufs=4)          # a'|b merged pads
        hab_pool = tc.alloc_tile_pool(name="hab", bufs=4)        # hab merged
        o_pool = tc.alloc_tile_pool(name="o", bufs=2)
        ps_pool = tc.alloc_tile_pool(name="ps", bufs=1, space="PSUM")
        psab_pool = tc.alloc_tile_pool(name="psab", bufs=2, space="PSUM")

        def mirrors(xp, off, eng):
            # left: cols 0:16 <- interior cols 32..17 (x[16..1]);
            # right: 16 cols (incl. pad col, keeps scan input finite).
            c0 = off + 16 + Wc
            eng.tensor_copy(xp[:, off : off + 16], xp[:, off + 32 : off + 16 : -1])
            eng.tensor_copy(xp[:, c0 : c0 + 16], xp[:, c0 - 2 : c0 - 18 : -1])

        def hscan(xp, lo, hi, out, dtag, eng, ieng, oo=0):
            # out[oo+w] = sum(xp[lo+w+1 .. lo+w+31]) for w in [0, hi-lo-31)
            n = hi - lo - 31
            init = cf_pool.tile([128, 1], f32, tag=f"init{dtag}", name=f"init{dtag}")
            ieng.reduce_sum(init[:], xp[:, lo : lo + 31], axis=AX)
            eng.tensor_tensor_scan(
                out[:, oo : oo + n], xp[:, lo + 31 : hi], xp[:, lo : lo + n], init[:],
                op0=OP.add, op1=OP.subtract,
            )

        def vpass(psum, hsrc, off, j, c):
            """psum[128, CH] = band-weighted column sums of hsrc block at off."""
            lo, hi = off + c * CH, off + (c + 1) * CH
            last_center = (j == 0 or hsrc[j - 1] is None) and (
                j == NT - 1 or hsrc[j + 1] is None
            )
            nc.tensor.matmul(
                psum[:], wv_sb[j][:, 0:128], hsrc[j][:, lo:hi],
                start=True, stop=last_center,
            )
            if j > 0 and hsrc[j - 1] is not None:
                nc.tensor.matmul(
                    psum[:], wv_sb[j][64:128, 128:256], hsrc[j - 1][64:128, lo:hi],
                    start=False, stop=(j == NT - 1 or hsrc[j + 1] is None),
                )
            if j < NT - 1 and hsrc[j + 1] is not None:
                nc.tensor.matmul(
                    psum[:], wv_sb[j][0:15, 256:384], hsrc[j + 1][0:15, lo:hi],
                    start=False, stop=True,
                )

        for img in range(n_img):
            xpI = [None] * NT
            hI = [None] * NT
            h3 = [None] * NT
            hab = [None] * NT

            def stageAB(j):
                _hp = tc.high_priority(offset=120)
                _hp.__enter__()
                xpI[j] = xpi_pool.tile([128, BS], bf16, tag="xpI", name="xpI")
                xp3 = xp3_pool.tile([128, 3 * BS], bf16, tag="xp3", name="xp3")
                nc.sync.dma_start(
                    xpI[j][:, 16 : 16 + Wc], gap[img, j * 128 : (j + 1) * 128, :]
                )
                nc.sync.dma_start(
                    xp3[:, 16 : 16 + Wc], pap[img, j * 128 : (j + 1) * 128, :]
                )
                mirrors(xpI[j], 0, nc.vector)
                mirrors(xp3, 0, nc.vector)
                # Ip block (mirrors inherited from I,p pads); II block via ACT
                nc.vector.tensor_mul(
                    xp3[:, BS : 2 * BS], xpI[j][:, 0:BS], xp3[:, 0:BS]
                )
                nc.scalar.activation(xp3[:, 2 * BS : 3 * BS], xpI[j][:, 0:BS], AF.Square)
                hI[j] = h_pool.tile([128, Wc], bf16, tag="hI", name="hI")
                h3[j] = h_pool.tile([128, 3 * BS - 31], bf16, tag="h3", name="h3")
                hscan(xpI[j], 0, BS - 1, hI[j], "I", nc.vector, nc.vector)
                # three separate Pool scans (finer grain than one merged scan,
                # avoids head-of-line blocking behind a 4.4us monolith)
                hscan(xp3, 0, BS + 15, h3[j], "3p", nc.gpsimd, nc.vector)
                hscan(xp3, BS, 2 * BS + 15, h3[j], "3q", nc.gpsimd, nc.vector, oo=BS)
                hscan(xp3, 2 * BS, 3 * BS, h3[j], "3r", nc.gpsimd, nc.vector, oo=2 * BS)
                _hp.__exit__(None, None, None)

            def stageCD(j):
                xpab = ab_pool.tile([128, 2 * BS], bf16, tag="xpab", name="xpab")
                # chunked V-passes + ACT evacuations into full-width SBUF tiles
                mI_s = cf_pool.tile([128, Wc], bf16, tag="mI_s", name="mI_s")
                mp_s = cf_pool.tile([128, Wc], bf16, tag="mp_s", name="mp_s")
                mIp_s = cf_pool.tile([128, Wc], bf16, tag="mIp_s", name="mIp_s")
                nII = cf_pool.tile([128, Wc], bf16, tag="nII", name="nII")
                for c in range(NC_):
                    mI = ps_pool.tile([128, CH], f32, tag="psA", name="psA")
                    mp = ps_pool.tile([128, CH], f32, tag="psB", name="psB")
                    mIp = ps_pool.tile([128, CH], f32, tag="psC", name="psC")
                    mII = ps_pool.tile([128, CH], f32, tag="psD", name="psD")
                    vpass(mI, hI, 0, j, c)
                    vpass(mp, h3, 0, j, c)
                    vpass(mIp, h3, BS, j, c)
                    vpass(mII, h3, 2 * BS, j, c)
                    cs = slice(c * CH, (c + 1) * CH)
                    nc.scalar.activation(mI_s[:, cs], mI[:], AF.Copy, scale=NORM)
                    nc.scalar.activation(mp_s[:, cs], mp[:], AF.Copy, scale=NORM)
                    nc.scalar.activation(mIp_s[:, cs], mIp[:], AF.Copy, scale=NORM)
                    nc.scalar.activation(nII[:, cs], mII[:], AF.Copy, scale=-NORM, bias=-EPS)
                # full-width elementwise chain (DVE 2x bf16 / Pool SBUF STT)
                prod = cf_pool.tile([128, Wc], bf16, tag="prod", name="prod")
                nc.vector.tensor_mul(prod[:], mI_s[:], mp_s[:])
                sq = cf_pool.tile([128, Wc], bf16, tag="sq", name="sq")
                nc.vector.tensor_mul(sq[:], mI_s[:], mI_s[:])
                cov = cf_pool.tile([128, Wc], bf16, tag="cov", name="cov")
                nc.gpsimd.scalar_tensor_tensor(
                    cov[:], mIp_s[:], 1.0, prod[:], op0=OP.mult, op1=OP.subtract
                )
                # negvar = -mean_II - EPS + mean_I^2 = -(var+eps)
                nvar = cf_pool.tile([128, Wc], f32, tag="nvar", name="nvar")
                nc.vector.scalar_tensor_tensor(
                    nvar[:], nII[:], 1.0, sq[:], op0=OP.mult, op1=OP.add
                )
                r = cf_pool.tile([128, Wc], f32, tag="r", name="r")
                nc.vector.reciprocal_approx_fast(out=r[:], in_=nvar[:])
                # a' = cov * r = -a  (Pool STT)
                av = xpab[:, 16 : 16 + Wc]
                nc.gpsimd.scalar_tensor_tensor(
                    av, cov[:], 1.0, r[:], op0=OP.mult, op1=OP.mult
                )
                # t' = a'*mean_I = -a*mean_I ; b = mean_p + t'
                t = cf_pool.tile([128, Wc], bf16, tag="t", name="t")
                nc.vector.tensor_mul(t[:], av, mI_s[:])
                nc.vector.tensor_add(xpab[:, BS + 16 : BS + 16 + Wc], mp_s[:], t[:])
                mirrors(xpab, 0, nc.gpsimd)
                mirrors(xpab, BS, nc.gpsimd)
                hab[j] = hab_pool.tile([128, 2 * BS - 31], bf16, tag="hab", name="hab")
                hscan(xpab, 0, BS + 15, hab[j], "a", nc.vector, nc.vector)
                hscan(xpab, BS, 2 * BS, hab[j], "b", nc.vector, nc.vector, oo=BS)

            def stageF(j):
                ot = o_pool.tile([128, Wc], bf16, tag="ot", name="ot")
                ma_s = o_pool.tile([128, Wc], bf16, tag="ma_s", name="ma_s")
                mb_s = o_pool.tile([128, Wc], bf16, tag="mb_s", name="mb_s")
                o1 = o_pool.tile([128, Wc], bf16, tag="o1", name="o1")
                for c in range(NC_):
                    ma = psab_pool.tile([128, CH], f32, tag="psa", name="psa")
                    mb = psab_pool.tile([128, CH], f32, tag="psb", name="psb")
                    vpass(ma, hab, 0, j, c)
                    vpass(mb, hab, BS, j, c)
                    cs = slice(c * CH, (c + 1) * CH)
                    nc.scalar.activation(ma_s[:, cs], ma[:], AF.Copy, scale=NORM)
                    nc.scalar.activation(mb_s[:, cs], mb[:], AF.Copy, scale=NORM)
                # o1 = mean_a' * I = -mean_a*I ; o = mean_b - o1
                nc.vector.tensor_mul(o1[:], ma_s[:], xpI[j][:, 16 : 16 + Wc])
                nc.gpsimd.scalar_tensor_tensor(
                    ot[:], mb_s[:], 1.0, o1[:], op0=OP.mult, op1=OP.subtract
                )
                nc.sync.dma_start(oap[img, j * 128 : (j + 1) * 128, :], ot[:])

            # software-pipelined emission: AB leads CD by 3 tiles, F lags CD by 1
            stageAB(0)
            if img == 0:
                # weight loads after the first tile's input DMAs so they don't
                # delay the pipeline head on the serial HWDGE/DMA devices
                for j in range(NT):
                    wt = wpool.tile([128, 384], bf16, tag=f"wv{j}", name=f"wv{j}")
                    nc.sync.dma_start(wt[:], wap[j])
                    wv_sb.append(wt)
            if NT > 1:
                stageAB(1)
            if NT > 2:
                stageAB(2)
            for j in range(NT):
                if j + 3 < NT:
                    stageAB(j + 3)
                stageCD(j)
                if j >= 1:
                    stageF(j - 1)
            stageF(NT - 1)

        for _pool in (psab_pool, ps_pool, o_pool, hab_pool, ab_pool, cf_pool,
                      h_pool, xp3_pool, xpi_pool, wpool):
            _pool.release()

    nc.compile()
    return nc


def _get_nc(n_img, Hc, Wc):
    key = (n_img, Hc, Wc)
    if key not in _CACHE:
        _CACHE[key] = build_nc(n_img, Hc, Wc)
    return _CACHE[key]


def kernel(guide, input_map):
    from concourse.bass_utils import run_bass_kernel_spmd

    B, C, Hc, Wc = guide.shape
    n_cores = 8
    n_img = B // n_cores
    bf16 = ml_dtypes.bfloat16
    g = np.ascontiguousarray(guide.reshape(B, Hc, Wc)).astype(bf16)
    p = np.ascontiguousarray(input_map.reshape(B, Hc, Wc)).astype(bf16)
    wv = _build_band_weights(Hc, Hc // 128)
    nc = _get_nc(n_img, Hc, Wc)
    in_maps = [
        {
            "guide": g[i * n_img : (i + 1) * n_img],
            "input_map": p[i * n_img : (i + 1) * n_img],
            "wv": wv,
        }
        for i in range(n_cores)
    ]
    res = run_bass_kernel_spmd(nc, in_maps, core_ids=list(range(n_cores)))
    out = np.concatenate([res.results[i]["out"] for i in range(n_cores)], axis=0)
    return out.reshape(B, C, Hc, Wc).astype(np.float32)
